# revision 1
# baseline (speedup 1.0000x reference)
import os
import sys
from contextlib import ExitStack

import numpy as np

for _p in (
    "/root/.axon_site",
    "/root/.axon_site/_ro/trn_rl_repo",
    "/root/.axon_site/_ro/pypackages",
    "/opt/trn_rl_repo",
):
    if os.path.isdir(_p) and _p not in sys.path:
        sys.path.append(_p)

import concourse.bass as bass
import concourse.mybir as mybir
from concourse import bass_utils

B, D = 8192, 128
N_CORES = 8
ROWS = B // N_CORES
S = ROWS // 128
H = S // 2
TEMPERATURE = 0.1
MARGIN = 0.5

_cache = {}


def _build():
    f32 = mybir.dt.float32
    mult = mybir.AluOpType.mult
    X = mybir.AxisListType.X
    nc = bass.Bass()
    z1p = nc.declare_dram_parameter("z1c", [ROWS, D], f32, isOutput=False)
    z2p = nc.declare_dram_parameter("z2c", [ROWS, D], f32, isOutput=False)
    outp = nc.declare_dram_parameter("partial", [128, 1], f32, isOutput=True)

    z1_ap = z1p[:].rearrange("(p s) d -> p s d", p=128)
    z2_ap = z2p[:].rearrange("(p s) d -> p s d", p=128)

    with ExitStack() as ctx:
        z1t = ctx.enter_context(nc.sbuf_tensor([128, S * D], f32))
        z2t = ctx.enter_context(nc.sbuf_tensor([128, S * D], f32))
        z1sq = ctx.enter_context(nc.sbuf_tensor([128, S * D], f32))
        vscr = ctx.enter_context(nc.sbuf_tensor([128, S * D], f32))
        gscr = ctx.enter_context(nc.sbuf_tensor([128, S * D], f32))
        dots = ctx.enter_context(nc.sbuf_tensor([128, S], f32))
        n1 = ctx.enter_context(nc.sbuf_tensor([128, S], f32))
        n2 = ctx.enter_context(nc.sbuf_tensor([128, S], f32))
        nsq = ctx.enter_context(nc.sbuf_tensor([128, S], f32))
        nrm = ctx.enter_context(nc.sbuf_tensor([128, S], f32))
        rec = ctx.enter_context(nc.sbuf_tensor([128, S], f32))
        pos = ctx.enter_context(nc.sbuf_tensor([128, S], f32))
        rowsum = ctx.enter_context(nc.sbuf_tensor([128, 1], f32))
        wtile = ctx.enter_context(nc.sbuf_tensor([128, 1], f32))
        z1a_sem = ctx.enter_context(nc.semaphore("z1a_sem"))
        z1b_sem = ctx.enter_context(nc.semaphore("z1b_sem"))
        z2a_sem = ctx.enter_context(nc.semaphore("z2a_sem"))
        z2b_sem = ctx.enter_context(nc.semaphore("z2b_sem"))
        st_sem = ctx.enter_context(nc.semaphore("st_sem"))
        act_sem = ctx.enter_context(nc.semaphore("act_sem"))
        dve_sem = ctx.enter_context(nc.semaphore("dve_sem"))
        done_sem = ctx.enter_context(nc.semaphore("done_sem"))
        block = ctx.enter_context(nc.Block())

        ones = nc.const_aps.scalar_like(1.0, wtile[:, :])

        def dot_group(s):
            return nc.vector.scalar_tensor_tensor(
                out=vscr[:, s * D : (s + 1) * D],
                in0=z1t[:, s * D : (s + 1) * D],
                scalar=1.0,
                in1=z2t[:, s * D : (s + 1) * D],
                op0=mult,
                op1=mult,
                accum_out=dots[:, s : s + 1],
            )

        @block.sync
        def _(sync):
            sync.dma_start(out=z1t[:, : H * D], in_=z1_ap[:, :H, :]).then_inc(
                z1a_sem, 16
            )
            sync.dma_start(out=z2t[:, : H * D], in_=z2_ap[:, :H, :]).then_inc(
                z2a_sem, 16
            )
            sync.dma_start(out=z1t[:, H * D :], in_=z1_ap[:, H:, :]).then_inc(
                z1b_sem, 16
            )
            sync.dma_start(out=z2t[:, H * D :], in_=z2_ap[:, H:, :]).then_inc(
                z2b_sem, 16
            )
            sync.wait_ge(done_sem, 1)
            sync.dma_start(out=outp[:], in_=rowsum[:, :]).then_inc(st_sem, 16)

        @block.scalar
        def _(scalar):
            nc.scalar.square(wtile[:, :], ones).then_inc(act_sem, 1)
            scalar.wait_ge(z1a_sem, 16)
            nc.scalar.square(z1sq[:, : H * D], z1t[:, : H * D]).then_inc(act_sem, 1)
            scalar.wait_ge(z2a_sem, 16)
            nc.scalar.square(gscr[:, : H * D], z2t[:, : H * D]).then_inc(act_sem, 1)
            scalar.wait_ge(z1b_sem, 16)
            nc.scalar.square(z1sq[:, H * D :], z1t[:, H * D :]).then_inc(act_sem, 1)
            scalar.wait_ge(z2b_sem, 16)
            nc.scalar.square(gscr[:, H * D :], z2t[:, H * D :]).then_inc(act_sem, 1)
            scalar.wait_ge(dve_sem, 13)
            nc.scalar.sqrt(nrm[:, :], nsq[:, :]).then_inc(act_sem, 1)

        @block.vector
        def _(vector):
            vector.wait_ge(z1a_sem, 16)
            vector.wait_ge(z2a_sem, 16)
            for s in range(H):
                dot_group(s).then_inc(dve_sem, 1)
            vector.wait_ge(act_sem, 2)
            nc.vector.reduce_sum(
                n1[:, :H],
                z1sq[:, : H * D].rearrange("p (s d) -> p s d", d=D),
                axis=X,
            ).then_inc(dve_sem, 1)
            vector.wait_ge(act_sem, 3)
            nc.vector.reduce_sum(
                n2[:, :H],
                gscr[:, : H * D].rearrange("p (s d) -> p s d", d=D),
                axis=X,
            ).then_inc(dve_sem, 1)
            vector.wait_ge(z1b_sem, 16)
            vector.wait_ge(z2b_sem, 16)
            for s in range(H, S):
                dot_group(s).then_inc(dve_sem, 1)
            vector.wait_ge(act_sem, 4)
            nc.vector.reduce_sum(
                n1[:, H:],
                z1sq[:, H * D :].rearrange("p (s d) -> p s d", d=D),
                axis=X,
            ).then_inc(dve_sem, 1)
            vector.wait_ge(act_sem, 5)
            nc.vector.reduce_sum(
                n2[:, H:],
                gscr[:, H * D :].rearrange("p (s d) -> p s d", d=D),
                axis=X,
            ).then_inc(dve_sem, 1)
            vector.wait_ge(dve_sem, 12)
            nc.vector.tensor_mul(nsq[:, :], n1[:, :], n2[:, :]).then_inc(dve_sem, 1)
            vector.wait_ge(act_sem, 6)
            nc.vector.reciprocal(rec[:, :], nrm[:, :]).then_inc(dve_sem, 1)
            vector.wait_ge(dve_sem, 14)
            nc.vector.scalar_tensor_tensor(
                out=pos[:, :],
                in0=dots[:, :],
                scalar=1.0,
                in1=rec[:, :],
                op0=mult,
                op1=mult,
                accum_out=rowsum[:, :],
            ).then_inc(done_sem, 1)

    return nc


def kernel(z1: np.ndarray, z2: np.ndarray) -> np.ndarray:
    z1 = np.ascontiguousarray(np.asarray(z1, dtype=np.float32))
    z2 = np.ascontiguousarray(np.asarray(z2, dtype=np.float32))
    assert z1.shape == (B, D) and z2.shape == (B, D)

    if "nc" not in _cache:
        _cache["nc"] = _build()
    nc = _cache["nc"]

    core_ids = list(range(N_CORES))
    in_maps = [
        {
            "z1c": z1[c * ROWS : (c + 1) * ROWS],
            "z2c": z2[c * ROWS : (c + 1) * ROWS],
        }
        for c in core_ids
    ]
    res = bass_utils.run_bass_kernel_spmd(nc, in_maps, core_ids)
    total = np.float64(0.0)
    for c in core_ids:
        total += np.sum(res.results[c]["partial"].astype(np.float64))
    loss = 1.0 / TEMPERATURE + MARGIN - total / float(B)
    return np.asarray(loss, dtype=np.float32)

